# revision 5
# baseline (speedup 1.0000x reference)
"""Trainium2 Bass kernel: ConvAttnPool + concept embeds (CAML-style label attention).

Sharding: pure data-parallel over batch B=8 across the 8 NeuronCores.
Core b computes the full pipeline for batch item b:
  gather/select embeds -> conv1d(tanh) -> label-attention softmax -> m -> y ->
  yhat/loss, plus the full [Y, L+1] normalized attention matrix (alpha).

Per-core device algorithm (all fp32):
  - indirect-DMA gathers of embed/concept rows; mask-select on DVE;
    PE transposes assemble zinT [E, L+2*PAD].
  - conv as 10 shifted matmuls accumulated in PSUM; tanh+bias fused on ACT
    -> zT [F, L+1]; PE transposes give z chunks with a ones column at index 64.
  - pass 1: scoresT tiles [l',y] on PE -> exp on ACT -> matmul against
    [z | 0.. | 1] computes m^T and the softmax row sums in one stream.
  - tiny PE matmuls transpose row sums and yu into [128, 70] column layout
    (value for label y at [y % 128, y // 128]); the y/yhat/loss stage runs
    there on small tiles; yhat is transposed back per 128-chunk for output.
  - pass 2 recomputes scores [y,l'] and applies exp(score - ln(sum)) via the
    ACT per-partition bias -> normalized alpha in a single ACT pass -> DMA out.
"""

import sys

sys.path.insert(0, "/opt/trn_rl_repo")

import numpy as np

B, L, E, F, K, Y = 8, 2500, 100, 50, 10, 8921
VOCAB, CVOCAB = 50002, 2002
PAD = K // 2
LP = L + 1          # conv output length, 2501
LPAD = L + 2 * PAD  # padded conv input length, 2510
N_CORES = 8
LT = 20             # l' chunks of 128 (19*128 + 69); input-l chunks (19*128 + 68)
YT = 70             # y tiles of 128 (69*128 + 89)
YG = 18             # y groups of 512 (17*512 + 217)
YPAD = YT * 128     # 8960
ZS = 65             # zones chunk stride; ones column lives at index 64

_NC = None


def _build_nc():
    import concourse.bacc as bacc
    import concourse.bass as bass
    import concourse.mybir as mybir
    import concourse.tile as tile
    from concourse.masks import make_identity

    f32 = mybir.dt.float32
    i32 = mybir.dt.int32
    Act = mybir.ActivationFunctionType
    Alu = mybir.AluOpType

    nc = bacc.Bacc("TRN2", target_bir_lowering=False, debug=False,
                   num_devices=N_CORES)

    x_col_d = nc.dram_tensor("x_col", [128, LT], i32, kind="ExternalInput")
    c_col_d = nc.dram_tensor("c_col", [128, LT], i32, kind="ExternalInput")
    embed_d = nc.dram_tensor("embed_w", [VOCAB, E], f32, kind="ExternalInput")
    concept_d = nc.dram_tensor("concept_w", [CVOCAB, E], f32, kind="ExternalInput")
    convwT_d = nc.dram_tensor("convw_t", [E, K * F], f32, kind="ExternalInput")
    convb_d = nc.dram_tensor("conv_b", [F], f32, kind="ExternalInput")
    uwT_d = nc.dram_tensor("uw_t", [F, Y], f32, kind="ExternalInput")
    finalT_d = nc.dram_tensor("final_t", [F, Y], f32, kind="ExternalInput")
    fbcol_d = nc.dram_tensor("final_b_col", [128, YT], f32, kind="ExternalInput")
    tgtcol_d = nc.dram_tensor("target_col", [128, YT], f32, kind="ExternalInput")
    alpha_d = nc.dram_tensor("alpha", [Y, LP], f32, kind="ExternalOutput")
    yhat_d = nc.dram_tensor("yhat", [Y], f32, kind="ExternalOutput")
    loss_d = nc.dram_tensor("loss_sum", [1], f32, kind="ExternalOutput")

    with tile.TileContext(nc, num_cores=N_CORES) as tc:
        with (
            tc.tile_pool(name="const", bufs=1) as cp,
            tc.tile_pool(name="work", bufs=3) as wp,
            tc.tile_pool(name="alpha_pool", bufs=2) as apool,
        ):
            uwT = cp.tile([F, Y], f32, name="uwT")
            finalT = cp.tile([F, YPAD], f32, name="finalT")
            convwT = cp.tile([E, K * F], f32, name="convwT")
            convb = cp.tile([F, 1], f32, name="convb")
            ident = cp.tile([128, 128], f32, name="ident")
            zinT = cp.tile([E, LPAD], f32, name="zinT")
            zT = cp.tile([F, LP], f32, name="zT")
            zones = cp.tile([128, ZS * LT], f32, name="zones")
            xcol = cp.tile([128, LT], i32, name="xcol")
            ccol = cp.tile([128, LT], i32, name="ccol")
            ccolf = cp.tile([128, LT], f32, name="ccolf")
            maskc = cp.tile([128, LT], f32, name="maskc")
            one1 = cp.tile([1, 1], f32, name="one1")
            ones50 = cp.tile([F, 1], f32, name="ones50")
            ones128 = cp.tile([128, 1], f32, name="ones128")
            # column-layout [128, YT] vectors: value for label y at [y%128, y//128]
            scol = cp.tile([128, YT], f32, name="scol")
            negls = cp.tile([128, YT], f32, name="negls")
            recipc = cp.tile([128, YT], f32, name="recipc")
            yucol = cp.tile([128, YT], f32, name="yucol")
            tgtcol = cp.tile([128, YT], f32, name="tgtcol")
            fbcol = cp.tile([128, YT], f32, name="fbcol")
            ycol = cp.tile([128, YT], f32, name="ycol")
            yhcol = cp.tile([128, YT], f32, name="yhcol")
            spcol = cp.tile([128, YT], f32, name="spcol")
            tmpc = cp.tile([128, YT], f32, name="tmpc")
            lelcol = cp.tile([128, YT], f32, name="lelcol")
            lredc = cp.tile([128, 1], f32, name="lredc")
            lsum = cp.tile([1, 1], f32, name="lsum")

            nc.sync.dma_start(out=uwT[:], in_=uwT_d[:])
            nc.sync.dma_start(out=finalT[:, 0:Y], in_=finalT_d[:])
            nc.sync.dma_start(out=convwT[:], in_=convwT_d[:])
            nc.sync.dma_start(out=convb[:], in_=convb_d[:].rearrange("(p o) -> p o", o=1))
            nc.sync.dma_start(out=xcol[:], in_=x_col_d[:])
            nc.sync.dma_start(out=ccol[:], in_=c_col_d[:])
            nc.sync.dma_start(out=tgtcol[:], in_=tgtcol_d[:])
            nc.sync.dma_start(out=fbcol[:], in_=fbcol_d[:])

            make_identity(nc, ident[:])
            nc.gpsimd.memset(one1[:], 1.0)
            nc.gpsimd.memset(ones50[:], 1.0)
            nc.gpsimd.memset(ones128[:], 1.0)
            nc.gpsimd.memset(finalT[:, Y:], 0.0)
            nc.gpsimd.memset(zones[:], 0.0)
            nc.gpsimd.memset(zinT[:, 0:PAD], 0.0)
            nc.gpsimd.memset(zinT[:, PAD + L:], 0.0)

            # concept-nonzero mask per token: min(concept_id, 1) in {0, 1}
            nc.vector.tensor_copy(out=ccolf[:], in_=ccol[:])
            nc.vector.tensor_scalar_min(out=maskc[:], in0=ccolf[:], scalar1=1.0)

            # ---- embedding gather + select + transpose into zinT [E, LPAD]
            with tc.tile_pool(name="pb", bufs=2, space="PSUM") as pb:
                for t in range(LT):
                    lcnt = 128 if t < LT - 1 else L - 128 * (LT - 1)
                    xe = wp.tile([128, E], f32, name="xe", tag="xe")
                    ce = wp.tile([128, E], f32, name="ce", tag="ce")
                    nc.gpsimd.indirect_dma_start(
                        out=xe[:], out_offset=None, in_=embed_d[:],
                        in_offset=bass.IndirectOffsetOnAxis(ap=xcol[:, t:t + 1], axis=0))
                    nc.gpsimd.indirect_dma_start(
                        out=ce[:], out_offset=None, in_=concept_d[:],
                        in_offset=bass.IndirectOffsetOnAxis(ap=ccol[:, t:t + 1], axis=0))
                    # zin = xe + mask * (ce - xe)
                    nc.vector.tensor_tensor(out=ce[:], in0=ce[:], in1=xe[:], op=Alu.subtract)
                    nc.vector.tensor_scalar_mul(out=ce[:], in0=ce[:], scalar1=maskc[:, t:t + 1])
                    nc.vector.tensor_tensor(out=ce[:], in0=ce[:], in1=xe[:], op=Alu.add)
                    tp = pb.tile([E, 128], f32, name="tp", tag="tp")
                    nc.tensor.transpose(out=tp[:E, :lcnt], in_=ce[:lcnt, :E],
                                        identity=ident[:lcnt, :lcnt])
                    nc.vector.tensor_copy(
                        out=zinT[:, PAD + 128 * t: PAD + 128 * t + lcnt], in_=tp[:E, :lcnt])

                # ---- conv1d as K shifted matmuls; tanh+bias fused on ACT
                for j in range(5):
                    w = 512 if j < 4 else LP - 2048
                    cps = pb.tile([F, 512], f32, name="cps", tag="cps")
                    for k in range(K):
                        nc.tensor.matmul(
                            out=cps[:F, :w], lhsT=convwT[:, F * k: F * k + F],
                            rhs=zinT[:, 512 * j + k: 512 * j + k + w],
                            start=(k == 0), stop=(k == K - 1))
                    nc.scalar.activation(out=zT[:, 512 * j: 512 * j + w], in_=cps[:F, :w],
                                         func=Act.Tanh, bias=convb[:F, :1], scale=1.0)

                # ---- z chunks in [l', f] layout, ones column at index 64
                for t in range(LT):
                    lw = 128 if t < LT - 1 else LP - 128 * (LT - 1)
                    nc.gpsimd.memset(zones[:lw, ZS * t + 64: ZS * t + 65], 1.0)
                    tz = pb.tile([128, F], f32, name="tz", tag="tp")
                    nc.tensor.transpose(out=tz[:lw, :F], in_=zT[:F, 128 * t: 128 * t + lw],
                                        identity=ident[:F, :F])
                    nc.vector.tensor_copy(out=zones[:lw, ZS * t: ZS * t + F], in_=tz[:lw, :F])

            # ---- pass 1: scoresT -> exp -> m^T (+ row sums via the ones column)
            with tc.tile_pool(name="pd", bufs=1, space="PSUM") as pd:
                scolp = pd.tile([128, YT], f32, name="scolp", tag="scolp")
                for gb in range(YG // 2):
                    y00 = 512 * (2 * gb)
                    y01 = 512 * (2 * gb + 1)
                    gw1 = 512 if 2 * gb + 1 < YG - 1 else Y - 512 * (YG - 1)
                    m0 = pd.tile([ZS, 512], f32, name="m0", tag="m0")
                    m1 = pd.tile([ZS, 512], f32, name="m1", tag="m1")
                    for t in range(LT):
                        lw = 128 if t < LT - 1 else LP - 128 * (LT - 1)
                        sct = pd.tile([128, 1024], f32, name="sct", tag="sct", bufs=2)
                        nc.tensor.matmul(out=sct[:lw, 0:512],
                                         lhsT=zT[:F, 128 * t: 128 * t + lw],
                                         rhs=uwT[:F, y00: y00 + 512],
                                         start=True, stop=True)
                        nc.tensor.matmul(out=sct[:lw, 512: 512 + gw1],
                                         lhsT=zT[:F, 128 * t: 128 * t + lw],
                                         rhs=uwT[:F, y01: y01 + gw1],
                                         start=True, stop=True)
                        ext = wp.tile([128, 1024], f32, name="ext", tag="ext")
                        nc.scalar.activation(out=ext[:lw, 0: 512 + gw1],
                                             in_=sct[:lw, 0: 512 + gw1], func=Act.Exp)
                        nc.tensor.matmul(out=m0[:ZS, 0:512],
                                         lhsT=zones[:lw, ZS * t: ZS * t + ZS],
                                         rhs=ext[:lw, 0:512],
                                         start=(t == 0), stop=(t == LT - 1))
                        nc.tensor.matmul(out=m1[:ZS, 0:gw1],
                                         lhsT=zones[:lw, ZS * t: ZS * t + ZS],
                                         rhs=ext[:lw, 512: 512 + gw1],
                                         start=(t == 0), stop=(t == LT - 1))
                    for (m, y0, gw) in ((m0, y00, 512), (m1, y01, gw1)):
                        # finalT slice becomes finalT * m^T (only product needed later)
                        nc.vector.tensor_tensor(out=finalT[:F, y0: y0 + gw],
                                                in0=finalT[:F, y0: y0 + gw],
                                                in1=m[:F, :gw], op=Alu.mult)
                        # stage the row-sum strip to SBUF (padded with 1.0), then
                        # transpose its 128-chunks into columns of scolp
                        srg = wp.tile([1, 512], f32, name="srg", tag="srg")
                        if gw < 512:
                            nc.gpsimd.memset(srg[0:1, gw:512], 1.0)
                        nc.vector.tensor_copy(out=srg[0:1, 0:gw], in_=m[64:65, :gw])
                        g = y0 // 512
                        for v in range((gw + 127) // 128):
                            u = 4 * g + v
                            nc.tensor.matmul(out=scolp[:, u:u + 1],
                                             lhsT=srg[0:1, 128 * v: 128 * v + 128],
                                             rhs=one1[:1, :1], start=True, stop=True)
                nc.vector.tensor_copy(out=scol[:], in_=scolp[:])

            # ---- column-layout y stage: yu, y, yhat, loss
            with tc.tile_pool(name="pe1", bufs=1, space="PSUM") as pe1:
                yucolp = pe1.tile([128, YT], f32, name="yucolp", tag="yucolp")
                for u in range(YT):
                    nc.tensor.matmul(out=yucolp[:, u:u + 1],
                                     lhsT=finalT[:F, 128 * u: 128 * u + 128],
                                     rhs=ones50[:F, :1], start=True, stop=True)
                nc.vector.tensor_copy(out=yucol[:], in_=yucolp[:])

            nc.scalar.activation(out=negls[:], in_=scol[:], func=Act.Ln)
            nc.vector.tensor_scalar_mul(out=negls[:], in0=negls[:], scalar1=-1.0)
            nc.scalar.activation(out=recipc[:], in_=negls[:], func=Act.Exp)
            nc.vector.tensor_tensor(out=ycol[:], in0=yucol[:], in1=recipc[:], op=Alu.mult)
            nc.vector.tensor_tensor(out=ycol[:], in0=ycol[:], in1=fbcol[:], op=Alu.add)
            nc.scalar.activation(out=yhcol[:], in_=ycol[:], func=Act.Sigmoid)
            # softplus(y) = ln(1 + exp(y)); loss elts = softplus(y) - t*y
            nc.scalar.activation(out=spcol[:], in_=ycol[:], func=Act.Exp)
            nc.vector.tensor_scalar_add(out=spcol[:], in0=spcol[:], scalar1=1.0)
            nc.scalar.activation(out=tmpc[:], in_=spcol[:], func=Act.Ln)
            nc.vector.tensor_tensor(out=lelcol[:], in0=tgtcol[:], in1=ycol[:], op=Alu.mult)
            nc.vector.tensor_tensor(out=lelcol[:], in0=tmpc[:], in1=lelcol[:], op=Alu.subtract)
            # exclude the 39 pad lanes of the last column from the loss sum
            nc.vector.tensor_reduce(out=lredc[:, 0:1], in_=lelcol[:, 0:YT - 1],
                                    axis=mybir.AxisListType.X, op=Alu.add)
            nc.vector.tensor_tensor(out=lredc[0:89, 0:1], in0=lredc[0:89, 0:1],
                                    in1=lelcol[0:89, YT - 1:YT], op=Alu.add)

            with tc.tile_pool(name="pe2", bufs=1, space="PSUM") as pe2:
                lp = pe2.tile([1, 1], f32, name="lp", tag="lp")
                nc.tensor.matmul(out=lp[:1, :1], lhsT=lredc[:, 0:1],
                                 rhs=ones128[:, 0:1], start=True, stop=True)
                nc.vector.tensor_copy(out=lsum[:], in_=lp[:])
                nc.sync.dma_start(out=loss_d[:].rearrange("(o n) -> o n", o=1),
                                  in_=lsum[0:1, 0:1])

                # yhat back to row layout, staged in 1280-wide strips
                for s in range(7):
                    ystage = wp.tile([1, 1280], f32, name="ystage", tag="ystage", bufs=2)
                    for q in range(10):
                        u = 10 * s + q
                        ytp = pe2.tile([1, 128], f32, name="ytp", tag="ytp", bufs=2)
                        nc.tensor.transpose(out=ytp[0:1, :128], in_=yhcol[:, u:u + 1],
                                            identity=ident[:])
                        nc.vector.tensor_copy(out=ystage[0:1, 128 * q: 128 * q + 128],
                                              in_=ytp[0:1, :128])
                    w = 1280 if s < 6 else Y - 7680
                    nc.sync.dma_start(
                        out=yhat_d[1280 * s: 1280 * s + w].rearrange("(o n) -> o n", o=1),
                        in_=ystage[0:1, 0:w])

                # ---- pass 2: alpha[y, l'] = exp(scores - ln(sums)) -> DRAM
                with tc.tile_pool(name="pf", bufs=1, space="PSUM") as pf:
                    for yt in range(YT):
                        tw = 128 if yt < YT - 1 else Y - 128 * (YT - 1)
                        yb = 128 * yt
                        spA = pf.tile([128, 2048], f32, name="spA", tag="spA")
                        spB = pf.tile([128, 512], f32, name="spB", tag="spB")
                        for j in range(4):
                            nc.tensor.matmul(out=spA[:tw, 512 * j: 512 * j + 512],
                                             lhsT=uwT[:F, yb: yb + tw],
                                             rhs=zT[:F, 512 * j: 512 * j + 512],
                                             start=True, stop=True)
                        nc.tensor.matmul(out=spB[:tw, 0: LP - 2048],
                                         lhsT=uwT[:F, yb: yb + tw],
                                         rhs=zT[:F, 2048:LP], start=True, stop=True)
                        at = apool.tile([128, 2560], f32, name="at", tag="at")
                        nc.scalar.activation(out=at[:tw, 0:2048], in_=spA[:tw, 0:2048],
                                             func=Act.Exp, bias=negls[:tw, yt:yt + 1],
                                             scale=1.0)
                        nc.scalar.activation(out=at[:tw, 2048:LP], in_=spB[:tw, 0: LP - 2048],
                                             func=Act.Exp, bias=negls[:tw, yt:yt + 1],
                                             scale=1.0)
                        nc.sync.dma_start(out=alpha_d[yb: yb + tw, :], in_=at[:tw, 0:LP])

    nc.compile()
    return nc


def _get_nc():
    global _NC
    if _NC is None:
        _NC = _build_nc()
    return _NC


def _colize_i32(v):
    p = np.zeros(128 * LT, np.int32)
    p[:L] = v
    return np.ascontiguousarray(p.reshape(LT, 128).T)


def _colize_f32(v):
    p = np.zeros(YPAD, np.float32)
    p[:Y] = v
    return np.ascontiguousarray(p.reshape(YT, 128).T)


def _prep_in_maps(inputs):
    x = np.asarray(inputs["x"]).astype(np.int32)
    con = np.asarray(inputs["concepts"]).astype(np.int32)
    tgt = np.asarray(inputs["target"], dtype=np.float32)
    embed = np.ascontiguousarray(np.asarray(inputs["embed_W"], dtype=np.float32))
    conw = np.ascontiguousarray(np.asarray(inputs["concept_W"], dtype=np.float32))
    convwT = np.ascontiguousarray(
        np.asarray(inputs["conv_w"], dtype=np.float32).transpose(1, 2, 0).reshape(E, K * F))
    convb = np.ascontiguousarray(np.asarray(inputs["conv_b"], dtype=np.float32))
    uwT = np.ascontiguousarray(np.asarray(inputs["U_w"], dtype=np.float32).T)
    finT = np.ascontiguousarray(np.asarray(inputs["final_w"], dtype=np.float32).T)
    fbcol = _colize_f32(np.asarray(inputs["final_b"], dtype=np.float32))

    in_maps = []
    for b in range(B):
        in_maps.append({
            "x_col": _colize_i32(x[b]),
            "c_col": _colize_i32(con[b]),
            "embed_w": embed,
            "concept_w": conw,
            "convw_t": convwT,
            "conv_b": convb,
            "uw_t": uwT,
            "final_t": finT,
            "final_b_col": fbcol,
            "target_col": _colize_f32(tgt[b]),
        })
    return in_maps


def kernel(**inputs):
    from concourse.bass_utils import run_bass_kernel_spmd

    nc = _get_nc()
    in_maps = _prep_in_maps(inputs)
    res = run_bass_kernel_spmd(nc, in_maps, list(range(N_CORES)))
    yhat = np.stack([np.asarray(res.results[b]["yhat"]) for b in range(B)])
    alpha = np.stack([np.asarray(res.results[b]["alpha"]) for b in range(B)])
    loss = np.float32(
        sum(float(np.asarray(res.results[b]["loss_sum"])[0]) for b in range(B)) / (B * Y))
    return yhat, loss, alpha


# revision 11
# speedup vs baseline: 1.5906x; 1.5906x over previous
"""Trainium2 Bass kernel: ConvAttnPool + concept embeds (CAML-style label attention).

Sharding: pure data-parallel over batch B=8 across the 8 NeuronCores.
Core b computes the full pipeline for batch item b:
  gather/select embeds -> conv1d(tanh) -> label-attention softmax -> m -> y ->
  yhat/loss, plus the full [Y, L+1] normalized attention matrix (alpha).

Per-core device algorithm (all fp32):
  - indirect-DMA gathers of embed/concept rows; mask-select on DVE;
    PE transposes assemble zinT [E, L+2*PAD].
  - conv as 10 shifted matmuls accumulated in PSUM; tanh+bias fused on ACT
    -> zT [F, L+1]; PE transposes give z chunks with a ones column at index 64.
  - pass 1: scoresT tiles [l',y] on PE -> exp on ACT -> matmul against
    [z | 0.. | 1] computes m^T and the softmax row sums in one stream.
  - tiny PE matmuls transpose row sums and yu into [128, 70] column layout
    (value for label y at [y % 128, y // 128]); the y/yhat/loss stage runs
    there on small tiles; yhat is transposed back per 128-chunk for output.
  - pass 2 recomputes scores [y,l'] and applies exp(score - ln(sum)) via the
    ACT per-partition bias -> normalized alpha in a single ACT pass -> DMA out.
"""

import sys

sys.path.insert(0, "/opt/trn_rl_repo")

import numpy as np

B, L, E, F, K, Y = 8, 2500, 100, 50, 10, 8921
VOCAB, CVOCAB = 50002, 2002
PAD = K // 2
LP = L + 1          # conv output length, 2501
LPAD = L + 2 * PAD  # padded conv input length, 2510
N_CORES = 8
LT = 20             # l' chunks of 128 (19*128 + 69); input-l chunks (19*128 + 68)
YT = 70             # y tiles of 128 (69*128 + 89)
YG = 18             # y groups of 512 (17*512 + 217)
YPAD = YT * 128     # 8960
ZS = 65             # zones chunk stride; ones column lives at index 64

_NC = None


def _build_nc():
    import concourse.bacc as bacc
    import concourse.bass as bass
    import concourse.mybir as mybir
    import concourse.tile as tile
    from concourse.masks import make_identity

    f32 = mybir.dt.float32
    f32r = mybir.dt.float32r
    i32 = mybir.dt.int32
    Act = mybir.ActivationFunctionType
    Alu = mybir.AluOpType

    nc = bacc.Bacc("TRN2", target_bir_lowering=False, debug=False,
                   num_devices=N_CORES)

    x_col_d = nc.dram_tensor("x_col", [128, LT], i32, kind="ExternalInput")
    c_col_d = nc.dram_tensor("c_col", [128, LT], i32, kind="ExternalInput")
    embed_d = nc.dram_tensor("embed_w", [VOCAB, E], f32, kind="ExternalInput")
    concept_d = nc.dram_tensor("concept_w", [CVOCAB, E], f32, kind="ExternalInput")
    convwT_d = nc.dram_tensor("convw_t", [E, K * F], f32, kind="ExternalInput")
    convb_d = nc.dram_tensor("conv_b", [F], f32, kind="ExternalInput")
    uwT_d = nc.dram_tensor("uw_t", [F, Y], f32, kind="ExternalInput")
    finalT_d = nc.dram_tensor("final_t", [F, Y], f32, kind="ExternalInput")
    fbcol_d = nc.dram_tensor("final_b_col", [128, YT], f32, kind="ExternalInput")
    tgtcol_d = nc.dram_tensor("target_col", [128, YT], f32, kind="ExternalInput")
    alpha_d = nc.dram_tensor("alpha", [Y, LP], f32, kind="ExternalOutput")
    yhat_d = nc.dram_tensor("yhat", [Y], f32, kind="ExternalOutput")
    loss_d = nc.dram_tensor("loss_sum", [1], f32, kind="ExternalOutput")

    with tile.TileContext(nc, num_cores=N_CORES) as tc:
        with (
            tc.tile_pool(name="const", bufs=1) as cp,
            tc.tile_pool(name="work", bufs=3) as wp,
            tc.tile_pool(name="alpha_pool", bufs=2) as apool,
        ):
            uwT = cp.tile([F, Y + 1], f32r, name="uwT")
            finalT = cp.tile([F, YPAD], f32, name="finalT")
            convwT = cp.tile([E, K * F], f32, name="convwT")
            convb = cp.tile([F, 1], f32, name="convb")
            ident = cp.tile([128, 128], f32, name="ident")
            identr = cp.tile([F, F], f32r, name="identr")
            ztmpl = cp.tile([128, 15], f32, name="ztmpl")
            zpad4 = cp.tile([F, 4], f32, name="zpad4")
            zinT = cp.tile([E, LPAD], f32, name="zinT")
            zT = cp.tile([F, LP + 3], f32r, name="zT")
            zones = cp.tile([128, ZS * LT], f32r, name="zones")
            xcol = cp.tile([128, LT], i32, name="xcol")
            ccol = cp.tile([128, LT], i32, name="ccol")
            ccolf = cp.tile([128, LT], f32, name="ccolf")
            maskc = cp.tile([128, LT], f32, name="maskc")
            one1 = cp.tile([1, 1], f32, name="one1")
            ones50 = cp.tile([F, 1], f32, name="ones50")
            ones128 = cp.tile([128, 1], f32, name="ones128")
            # column-layout [128, YT] vectors: value for label y at [y%128, y//128]
            scol = cp.tile([128, YT], f32, name="scol")
            negls = cp.tile([128, YT], f32, name="negls")
            recipc = cp.tile([128, YT], f32, name="recipc")
            yucol = cp.tile([128, YT], f32, name="yucol")
            tgtcol = cp.tile([128, YT], f32, name="tgtcol")
            fbcol = cp.tile([128, YT], f32, name="fbcol")
            ycol = cp.tile([128, YT], f32, name="ycol")
            yhcol = cp.tile([128, YT], f32, name="yhcol")
            spcol = cp.tile([128, YT], f32, name="spcol")
            tmpc = cp.tile([128, YT], f32, name="tmpc")
            lelcol = cp.tile([128, YT], f32, name="lelcol")
            lredc = cp.tile([128, 1], f32, name="lredc")
            lsum = cp.tile([1, 1], f32, name="lsum")

            nc.sync.dma_start(out=uwT[:, 0:Y], in_=uwT_d[:].bitcast(f32r))
            nc.sync.dma_start(out=finalT[:, 0:Y], in_=finalT_d[:])
            nc.sync.dma_start(out=convwT[:], in_=convwT_d[:])
            nc.sync.dma_start(out=convb[:], in_=convb_d[:].rearrange("(p o) -> p o", o=1))
            nc.sync.dma_start(out=xcol[:], in_=x_col_d[:])
            nc.sync.dma_start(out=ccol[:], in_=c_col_d[:])
            nc.sync.dma_start(out=tgtcol[:], in_=tgtcol_d[:])
            nc.sync.dma_start(out=fbcol[:], in_=fbcol_d[:])

            make_identity(nc, ident[:])
            nc.vector.tensor_copy(out=identr[:], in_=ident[:F, :F])
            nc.gpsimd.memset(one1[:], 1.0)
            nc.gpsimd.memset(ones50[:], 1.0)
            nc.gpsimd.memset(ones128[:], 1.0)
            nc.gpsimd.memset(finalT[:, Y:], 0.0)
            nc.gpsimd.memset(zinT[:, 0:PAD], 0.0)
            nc.gpsimd.memset(zinT[:, PAD + L:], 0.0)
            nc.gpsimd.memset(ztmpl[:], 0.0)
            nc.gpsimd.memset(ztmpl[:, 14:15], 1.0)
            nc.gpsimd.memset(zpad4[:], 0.0)
            # zero-fill the even-width pads of the f32r operands
            nc.vector.tensor_copy(out=uwT[:, Y:Y + 1], in_=zpad4[:F, 0:1])
            nc.vector.tensor_copy(out=zT[:, LP:LP + 3], in_=zpad4[:F, 0:3])

            # concept-nonzero mask per token: min(concept_id, 1) in {0, 1}
            nc.vector.tensor_copy(out=ccolf[:], in_=ccol[:])
            nc.vector.tensor_scalar_min(out=maskc[:], in0=ccolf[:], scalar1=1.0)

            # ---- embedding gather + select + transpose into zinT [E, LPAD]
            with tc.tile_pool(name="pb", bufs=2, space="PSUM") as pb:
                for t in range(LT):
                    lcnt = 128 if t < LT - 1 else L - 128 * (LT - 1)
                    xe = wp.tile([128, E], f32, name="xe", tag="xe")
                    ce = wp.tile([128, E], f32, name="ce", tag="ce")
                    nc.gpsimd.indirect_dma_start(
                        out=xe[:], out_offset=None, in_=embed_d[:],
                        in_offset=bass.IndirectOffsetOnAxis(ap=xcol[:, t:t + 1], axis=0))
                    nc.gpsimd.indirect_dma_start(
                        out=ce[:], out_offset=None, in_=concept_d[:],
                        in_offset=bass.IndirectOffsetOnAxis(ap=ccol[:, t:t + 1], axis=0))
                    # zin = xe + mask * (ce - xe)
                    nc.vector.tensor_tensor(out=ce[:], in0=ce[:], in1=xe[:], op=Alu.subtract)
                    nc.vector.tensor_scalar_mul(out=ce[:], in0=ce[:], scalar1=maskc[:, t:t + 1])
                    nc.vector.tensor_tensor(out=ce[:], in0=ce[:], in1=xe[:], op=Alu.add)
                    tp = pb.tile([E, 128], f32, name="tp", tag="tp")
                    nc.tensor.transpose(out=tp[:E, :lcnt], in_=ce[:lcnt, :E],
                                        identity=ident[:lcnt, :lcnt])
                    nc.vector.tensor_copy(
                        out=zinT[:, PAD + 128 * t: PAD + 128 * t + lcnt], in_=tp[:E, :lcnt])

                # ---- conv1d as K shifted matmuls; tanh+bias fused on ACT
                for j in range(5):
                    w = 512 if j < 4 else LP - 2048
                    cps = pb.tile([F, 512], f32, name="cps", tag="cps")
                    for k in range(K):
                        nc.tensor.matmul(
                            out=cps[:F, :w], lhsT=convwT[:, F * k: F * k + F],
                            rhs=zinT[:, 512 * j + k: 512 * j + k + w],
                            start=(k == 0), stop=(k == K - 1))
                    nc.scalar.activation(out=zT[:, 512 * j: 512 * j + w], in_=cps[:F, :w],
                                         func=Act.Tanh, bias=convb[:F, :1], scale=1.0)

                # ---- z chunks in [l', f] layout, ones column at index 64
                for t in range(LT):
                    lw = 128 if t < LT - 1 else LP - 128 * (LT - 1)
                    nc.vector.tensor_copy(out=zones[:, ZS * t + 50: ZS * t + 65],
                                          in_=ztmpl[:])
                    tz = pb.tile([128, F], f32r, name="tz", tag="tp")
                    nc.tensor.transpose(out=tz[:lw, :F], in_=zT[:F, 128 * t: 128 * t + lw],
                                        identity=identr[:F, :F])
                    nc.vector.tensor_copy(out=zones[:lw, ZS * t: ZS * t + F], in_=tz[:lw, :F])

            # ---- pass 1: scoresT -> exp -> m^T (+ row sums via the ones column)
            with tc.tile_pool(name="pd", bufs=1, space="PSUM") as pd:
                scolp = pd.tile([128, YT], f32, name="scolp", tag="scolp")
                for gb in range(YG // 2):
                    y00 = 512 * (2 * gb)
                    y01 = 512 * (2 * gb + 1)
                    gw1 = 512 if 2 * gb + 1 < YG - 1 else Y - 512 * (YG - 1)
                    gw1e = gw1 + (gw1 % 2)
                    m0 = pd.tile([ZS, 512], f32, name="m0", tag="m0")
                    m1 = pd.tile([ZS, 512], f32, name="m1", tag="m1")
                    for t in range(LT):
                        lw = 128 if t < LT - 1 else LP - 128 * (LT - 1)
                        sct = pd.tile([128, 1024], f32, name="sct", tag="sct", bufs=2)
                        nc.tensor.matmul(out=sct[:lw, 0:512],
                                         lhsT=zT[:F, 128 * t: 128 * t + lw],
                                         rhs=uwT[:F, y00: y00 + 512],
                                         start=True, stop=True)
                        nc.tensor.matmul(out=sct[:lw, 512: 512 + gw1e],
                                         lhsT=zT[:F, 128 * t: 128 * t + lw],
                                         rhs=uwT[:F, y01: y01 + gw1e],
                                         start=True, stop=True)
                        ext = wp.tile([128, 1024], f32r, name="ext", tag="ext")
                        nc.scalar.activation(out=ext[:lw, 0: 512 + gw1e],
                                             in_=sct[:lw, 0: 512 + gw1e], func=Act.Exp)
                        nc.tensor.matmul(out=m0[:ZS, 0:512],
                                         lhsT=zones[:lw, ZS * t: ZS * t + ZS],
                                         rhs=ext[:lw, 0:512],
                                         start=(t == 0), stop=(t == LT - 1))
                        nc.tensor.matmul(out=m1[:ZS, 0:gw1e],
                                         lhsT=zones[:lw, ZS * t: ZS * t + ZS],
                                         rhs=ext[:lw, 512: 512 + gw1e],
                                         start=(t == 0), stop=(t == LT - 1))
                    for (m, y0, gw) in ((m0, y00, 512), (m1, y01, gw1)):
                        # finalT slice becomes finalT * m^T (only product needed later)
                        nc.vector.tensor_tensor(out=finalT[:F, y0: y0 + gw],
                                                in0=finalT[:F, y0: y0 + gw],
                                                in1=m[:F, :gw], op=Alu.mult)
                        # stage the row-sum strip to SBUF (padded with 1.0), then
                        # transpose its 128-chunks into columns of scolp
                        srg = wp.tile([1, 512], f32, name="srg", tag="srg")
                        if gw < 512:
                            nc.gpsimd.memset(srg[0:1, gw:512], 1.0)
                        nc.vector.tensor_copy(out=srg[0:1, 0:gw], in_=m[64:65, :gw])
                        g = y0 // 512
                        for v in range((gw + 127) // 128):
                            u = 4 * g + v
                            nc.tensor.matmul(out=scolp[:, u:u + 1],
                                             lhsT=srg[0:1, 128 * v: 128 * v + 128],
                                             rhs=one1[:1, :1], start=True, stop=True)
                nc.vector.tensor_copy(out=scol[:], in_=scolp[:])

            # ---- column-layout y stage: yu, y, yhat, loss
            with tc.tile_pool(name="pe1", bufs=1, space="PSUM") as pe1:
                yucolp = pe1.tile([128, YT], f32, name="yucolp", tag="yucolp")
                for u in range(YT):
                    nc.tensor.matmul(out=yucolp[:, u:u + 1],
                                     lhsT=finalT[:F, 128 * u: 128 * u + 128],
                                     rhs=ones50[:F, :1], start=True, stop=True)
                nc.vector.tensor_copy(out=yucol[:], in_=yucolp[:])

            nc.scalar.activation(out=negls[:], in_=scol[:], func=Act.Ln)
            nc.vector.tensor_scalar_mul(out=negls[:], in0=negls[:], scalar1=-1.0)
            nc.scalar.activation(out=recipc[:], in_=negls[:], func=Act.Exp)
            nc.vector.tensor_tensor(out=ycol[:], in0=yucol[:], in1=recipc[:], op=Alu.mult)
            nc.vector.tensor_tensor(out=ycol[:], in0=ycol[:], in1=fbcol[:], op=Alu.add)
            nc.scalar.activation(out=yhcol[:], in_=ycol[:], func=Act.Sigmoid)
            # softplus(y) = ln(1 + exp(y)); loss elts = softplus(y) - t*y
            nc.scalar.activation(out=spcol[:], in_=ycol[:], func=Act.Exp)
            nc.vector.tensor_scalar_add(out=spcol[:], in0=spcol[:], scalar1=1.0)
            nc.scalar.activation(out=tmpc[:], in_=spcol[:], func=Act.Ln)
            nc.vector.tensor_tensor(out=lelcol[:], in0=tgtcol[:], in1=ycol[:], op=Alu.mult)
            nc.vector.tensor_tensor(out=lelcol[:], in0=tmpc[:], in1=lelcol[:], op=Alu.subtract)
            # exclude the 39 pad lanes of the last column from the loss sum
            nc.vector.tensor_reduce(out=lredc[:, 0:1], in_=lelcol[:, 0:YT - 1],
                                    axis=mybir.AxisListType.X, op=Alu.add)
            nc.vector.tensor_tensor(out=lredc[0:89, 0:1], in0=lredc[0:89, 0:1],
                                    in1=lelcol[0:89, YT - 1:YT], op=Alu.add)

            with tc.tile_pool(name="pe2", bufs=1, space="PSUM") as pe2:
                lp = pe2.tile([1, 1], f32, name="lp", tag="lp")
                nc.tensor.matmul(out=lp[:1, :1], lhsT=lredc[:, 0:1],
                                 rhs=ones128[:, 0:1], start=True, stop=True)
                nc.vector.tensor_copy(out=lsum[:], in_=lp[:])
                nc.sync.dma_start(out=loss_d[:].rearrange("(o n) -> o n", o=1),
                                  in_=lsum[0:1, 0:1])

                # yhat back to row layout, staged in 1280-wide strips
                for s in range(7):
                    ystage = wp.tile([1, 1280], f32, name="ystage", tag="ystage", bufs=2)
                    for q in range(10):
                        u = 10 * s + q
                        ytp = pe2.tile([1, 128], f32, name="ytp", tag="ytp", bufs=2)
                        nc.tensor.transpose(out=ytp[0:1, :128], in_=yhcol[:, u:u + 1],
                                            identity=ident[:])
                        nc.vector.tensor_copy(out=ystage[0:1, 128 * q: 128 * q + 128],
                                              in_=ytp[0:1, :128])
                    w = 1280 if s < 6 else Y - 7680
                    nc.sync.dma_start(
                        out=yhat_d[1280 * s: 1280 * s + w].rearrange("(o n) -> o n", o=1),
                        in_=ystage[0:1, 0:w])

                # ---- pass 2: alpha[y, l'] = exp(scores - ln(sums)) -> DRAM
                with tc.tile_pool(name="pf", bufs=1, space="PSUM") as pf:
                    for yt in range(YT):
                        tw = 128 if yt < YT - 1 else Y - 128 * (YT - 1)
                        yb = 128 * yt
                        spA = pf.tile([128, 2048], f32, name="spA", tag="spA")
                        spB = pf.tile([128, 512], f32, name="spB", tag="spB")
                        for j in range(4):
                            nc.tensor.matmul(out=spA[:tw, 512 * j: 512 * j + 512],
                                             lhsT=uwT[:F, yb: yb + tw],
                                             rhs=zT[:F, 512 * j: 512 * j + 512],
                                             start=True, stop=True)
                        nc.tensor.matmul(out=spB[:tw, 0: LP + 1 - 2048],
                                         lhsT=uwT[:F, yb: yb + tw],
                                         rhs=zT[:F, 2048:LP + 1],
                                         start=True, stop=True)
                        at = apool.tile([128, 2560], f32, name="at", tag="at")
                        nc.scalar.activation(out=at[:tw, 0:2048], in_=spA[:tw, 0:2048],
                                             func=Act.Exp, bias=negls[:tw, yt:yt + 1],
                                             scale=1.0)
                        nc.scalar.activation(out=at[:tw, 2048:LP], in_=spB[:tw, 0: LP - 2048],
                                             func=Act.Exp, bias=negls[:tw, yt:yt + 1],
                                             scale=1.0)
                        nc.sync.dma_start(out=alpha_d[yb: yb + tw, :], in_=at[:tw, 0:LP])

    nc.compile()
    return nc


def _get_nc():
    global _NC
    if _NC is None:
        _NC = _build_nc()
    return _NC


def _colize_i32(v):
    p = np.zeros(128 * LT, np.int32)
    p[:L] = v
    return np.ascontiguousarray(p.reshape(LT, 128).T)


def _colize_f32(v):
    p = np.zeros(YPAD, np.float32)
    p[:Y] = v
    return np.ascontiguousarray(p.reshape(YT, 128).T)


def _prep_in_maps(inputs):
    x = np.asarray(inputs["x"]).astype(np.int32)
    con = np.asarray(inputs["concepts"]).astype(np.int32)
    tgt = np.asarray(inputs["target"], dtype=np.float32)
    embed = np.ascontiguousarray(np.asarray(inputs["embed_W"], dtype=np.float32))
    conw = np.ascontiguousarray(np.asarray(inputs["concept_W"], dtype=np.float32))
    convwT = np.ascontiguousarray(
        np.asarray(inputs["conv_w"], dtype=np.float32).transpose(1, 2, 0).reshape(E, K * F))
    convb = np.ascontiguousarray(np.asarray(inputs["conv_b"], dtype=np.float32))
    uwT = np.ascontiguousarray(np.asarray(inputs["U_w"], dtype=np.float32).T)
    finT = np.ascontiguousarray(np.asarray(inputs["final_w"], dtype=np.float32).T)
    fbcol = _colize_f32(np.asarray(inputs["final_b"], dtype=np.float32))

    in_maps = []
    for b in range(B):
        in_maps.append({
            "x_col": _colize_i32(x[b]),
            "c_col": _colize_i32(con[b]),
            "embed_w": embed,
            "concept_w": conw,
            "convw_t": convwT,
            "conv_b": convb,
            "uw_t": uwT,
            "final_t": finT,
            "final_b_col": fbcol,
            "target_col": _colize_f32(tgt[b]),
        })
    return in_maps


def kernel(**inputs):
    from concourse.bass_utils import run_bass_kernel_spmd

    nc = _get_nc()
    in_maps = _prep_in_maps(inputs)
    res = run_bass_kernel_spmd(nc, in_maps, list(range(N_CORES)))
    yhat = np.stack([np.asarray(res.results[b]["yhat"]) for b in range(B)])
    alpha = np.stack([np.asarray(res.results[b]["alpha"]) for b in range(B)])
    loss = np.float32(
        sum(float(np.asarray(res.results[b]["loss_sum"])[0]) for b in range(B)) / (B * Y))
    return yhat, loss, alpha
